# revision 1
# baseline (speedup 1.0000x reference)
"""
MinibatchDiscrimination kernel for 8x TRN2 NeuronCores (Bass/Tile).

Math:  x = inputs @ T  -> [B, K, D] with B=512, K=100, D=5
       out[i,k] = sum_j exp(-sum_d |x[i,k,d]-x[j,k,d]|)

Strategy (per core c of 8):
  - Host passes, per core: inputsT_c = (roll(inputs, -64c, axis=0)).T as fp16
    [F, B], T as fp16 [F, KD], plus small constant matrices. Rolling the
    batch axis makes the program SPMD-identical: every core computes output
    rows for "columns 0..63" of its own xT.
  - Device: xT[kd, i] = sum_f T[f, kd] * inputsT[f, i]   (PE, 4 chunks of 125)
    S[k, i] = sum_d x[i,k,d]  (PE ones-block matmul over xT, stored fp16)
  - Per output row j in 0..63, using |t| = 2*relu(t) - t:
      rl_c[p,i]   = relu(xT_c[p,i] - xT_c[p,j])   (DVE tensor_scalar
                                                   (subtract, max 0.0); the
                                                   per-partition scalar is an
                                                   f32 upcast of the fp16 xT
                                                   column so the diagonal is
                                                   exactly 0)
      dist[:, i]  = -S[k,i] + 2*sum_d rl           (PE: negI matmul into psum +
                                                    2.0-block col-tiled matmuls)
      raw[:, j]   = sum_i exp(-dist[:,i])          (ACT fused exp + accum_out,
                                                    no bias)
      out         = raw * exp(-S16[:, 0:64])       (one DVE multiply at the
                                                    end: the per-partition
                                                    exp(-S_kj) factor is
                                                    constant over i, so it
                                                    factors out of the sum)
    since sum_d |diff| = 2*sum_d relu(diff) - S_ki + S_kj, and the S terms
    cancel exactly on the diagonal.
  - dist row p=32c+m holds k=25c+m (m<25); host transposes/reassembles.

  Hardware notes baked into the structure (measured on TRN2):
  - Compute instructions carry at most ONE semaphore wait after bacc's
    split pass; persistent manually-rotated tiles (dist/dump/ab) keep
    cross-iteration WAR deps same-engine so waits stay within budget.
  - The pipeline is ACT/DVE-bound: exp+accum ~= 0.85us per row; DVE
    (4 tensor_scalars ~1us) and PE (~0.9us, partially col-tile-overlapped)
    overlap with it. Measured steady state ~64-72us for the 64-row loop
    (axon-link wall-clock noise is +-10us; best measured 63.8us).
    GPSIMD offload of a relu chunk was tried and is ~7x slower than DVE
    on the Q7 path -- do not route tensor_scalar to gpsimd here.
    Also measured as no-wins (within +-10us link noise): deeper ab/dist
    buffering (12/5), exp main-out to SBUF fp16 instead of PSUM f32,
    and a rank-1 PE matmul replacing the exp bias (that one regressed
    ~40% -- K=1 matmuls serialize on the PE critical path).
  - Residual overhead, quantified from the final IR: each relu
    tensor_scalar carries a redundant same-engine WAW wait (ab-buffer
    rotation) in addition to its real PE WAR; bacc's 1-wait limit splits
    it into an EventSemaphore on the DVE queue -- 242 of them, ~10us of
    issue time on the bottleneck engine. Eliding same-engine WAW sems in
    Tile/bacc would recover most of the gap to the ~60us arithmetic floor.
  - The input stage (DMA 2MB fp16, xT matmuls, S row-sums) overlaps the
    loop start; the ACT exp table is pre-warmed during the DMAs.

By symmetry of the distance matrix, summing exp(-dist) over the free axis i
for a fixed row j gives exactly out[j, k] (self term included).
"""

import sys
import numpy as np

for _p in ("/opt/trn_rl_repo",):
    if _p not in sys.path:
        sys.path.insert(0, _p)

B = 512
F = 1024
K = 100
D = 5
KD = K * D  # 500
NCORES = 8
JPC = B // NCORES  # 64 output rows per core
NCHUNK = 4  # kd chunks of 125
CHUNK = KD // NCHUNK  # 125
KPC = K // NCHUNK  # 25 k's per chunk

_NC_CACHE = {}


def build_nc(bench_reps=1, ablate=()):
    import contextlib

    import concourse.bass as bass
    import concourse.bacc as bacc
    import concourse.mybir as mybir
    from concourse.tile import TileContext

    nc = bacc.Bacc(None, target_bir_lowering=False, debug=True)

    inT = nc.declare_dram_parameter("inT", [F, B], mybir.dt.float16, isOutput=False)
    Tm = nc.declare_dram_parameter("Tm", [F, KD], mybir.dt.float16, isOutput=False)
    # [:, 0:32] 2.0-valued d-sum block, [:, 32:64] 1.0-valued d-sum block
    onesd = nc.declare_dram_parameter(
        "onesd", [CHUNK, 64], mybir.dt.float16, isOutput=False
    )
    negI = nc.declare_dram_parameter("negI", [128, 128], mybir.dt.float16, isOutput=False)
    out = nc.declare_dram_parameter("out", [128, JPC], mybir.dt.float32, isOutput=True)

    with TileContext(nc) as tc:
        with tc.tile_pool(name="persist", bufs=1) as pp:
            T_sb = pp.tile([128, 8 * KD], mybir.dt.float16, name="T_sb")
            inT_sb = pp.tile([128, 8 * B], mybir.dt.float16, name="inT_sb")
            ones_sb = pp.tile([CHUNK, 64], mybir.dt.float16, name="ones_sb")
            negI_sb = pp.tile([128, 128], mybir.dt.float16, name="negI_sb")
            out_sb = pp.tile([128, JPC], mybir.dt.float32, name="out_sb")
            xT_sb = pp.tile([128, NCHUNK * B], mybir.dt.float16, name="xT_sb")
            # f32 upcasts of xT columns 0..JPC (tensor_scalar per-partition
            # scalars must be f32). Upcast from the fp16 xT so the diagonal
            # max(x,x)-x stays exactly zero.
            xTj_sb = pp.tile([128, NCHUNK * JPC], mybir.dt.float32, name="xTj_sb")
            S16_sb = pp.tile([128, B], mybir.dt.float16, name="S16_sb")
            expS_sb = pp.tile([128, JPC], mybir.dt.float32, name="expS_sb")
            raw_sb = pp.tile([128, JPC], mybir.dt.float32, name="raw_sb")

            # warm the ACT exp table while DMAs run (table load ~2.7us)
            warm_sb = pp.tile([1, 1], mybir.dt.float32, name="warm_sb")
            nc.vector.memset(warm_sb[:, :], 0.0)
            nc.scalar.activation(
                warm_sb[:, :], warm_sb[:, :], mybir.ActivationFunctionType.Exp
            )

            # --- load inputs ---
            for t in range(8):
                nc.sync.dma_start(
                    out=T_sb[:, t * KD : (t + 1) * KD],
                    in_=Tm[t * 128 : (t + 1) * 128, :],
                )
                nc.sync.dma_start(
                    out=inT_sb[:, t * B : (t + 1) * B],
                    in_=inT[t * 128 : (t + 1) * 128, :],
                )
            nc.sync.dma_start(out=ones_sb[:, :], in_=onesd[:, :])
            nc.sync.dma_start(out=negI_sb[:, :], in_=negI[:, :])

            with tc.tile_pool(name="xtps", bufs=2, space="PSUM") as xtps:
                # --- xT chunks: xT[kd, i] via PE over f tiles ---
                for c in range(NCHUNK):
                    xt_ps = xtps.tile([CHUNK, B], mybir.dt.float32, name="xt_ps")
                    for t in range(8):
                        nc.tensor.matmul(
                            xt_ps[:, :],
                            T_sb[:, t * KD + c * CHUNK : t * KD + (c + 1) * CHUNK],
                            inT_sb[:, t * B : (t + 1) * B],
                            start=(t == 0),
                            stop=(t == 7),
                        )
                    nc.scalar.copy(xT_sb[0:CHUNK, c * B : (c + 1) * B], xt_ps[:, :])
                    nc.vector.tensor_copy(
                        xTj_sb[0:CHUNK, c * JPC : (c + 1) * JPC],
                        xT_sb[0:CHUNK, c * B : c * B + JPC],
                    )

                # --- S[k, i] = sum_d x[i,k,d], arranged at partitions 32c+m ---
                S_ps = xtps.tile([128, B], mybir.dt.float32, name="S_ps", bufs=1)
                for c in range(NCHUNK):
                    nc.tensor.matmul(
                        S_ps[32 * c : 32 * c + 32, :],
                        ones_sb[:, 32:64],
                        xT_sb[0:CHUNK, c * B : (c + 1) * B],
                        start=True,
                        stop=True,
                        tile_position=(0, 32 * c),
                    )
                nc.vector.tensor_copy(S16_sb[:, :], S_ps[:, :])
                # exp(-S16[:, j]) factors: the per-partition exp bias is
                # constant over i, so it moves out of the accumulated sum
                # and becomes one elementwise multiply at the end.
                nc.scalar.activation(
                    expS_sb[:, :],
                    S16_sb[:, 0:JPC],
                    mybir.ActivationFunctionType.Exp,
                    bias=0.0,
                    scale=-1.0,
                )

            mainps_es = contextlib.ExitStack()
            mainps = mainps_es.enter_context(
                tc.tile_pool(name="mainps", bufs=1, space="PSUM")
            )

            # Persistent, manually double-buffered psum tiles. Persistent
            # (vs pool-rotated) so cross-iteration WAR deps are plain data
            # deps on fixed tiles: same-engine deps then cost no semaphore,
            # which matters because instructions carry at most ONE wait.
            dist_bufs = [
                mainps.tile([128, B], mybir.dt.float32, name=f"dist{i}") for i in range(4)
            ]
            dump_bufs = [
                mainps.tile([128, B], mybir.dt.float32, name=f"dump{i}") for i in range(2)
            ]
            # Persistent relu tiles, manually rotated (same reason).
            NAB = 8
            ab_bufs = [
                pp.tile([CHUNK, B], mybir.dt.float16, name=f"ab{i}") for i in range(NAB)
            ]

            # --- main loop over output rows ---
            # bench_reps>1 wraps the j-loop in a dynamic For_i so one NEFF
            # execution repeats the steady-state body (timing harness only).
            if ablate:
                # one unablated pass so every tile has a writer
                main_loop(nc, mybir, xT_sb, xTj_sb, S16_sb, raw_sb, negI_sb,
                          ones_sb, out_sb, dist_bufs, dump_bufs, ab_bufs)
            rep_ctx = (
                tc.For_i(0, bench_reps, 1) if bench_reps > 1 else contextlib.nullcontext()
            )
            with rep_ctx:
                main_loop(nc, mybir, xT_sb, xTj_sb, S16_sb, raw_sb, negI_sb,
                          ones_sb, out_sb, dist_bufs, dump_bufs, ab_bufs, ablate)

            # out = raw_sums * exp(-S16[:, j]) (the factored-out bias)
            nc.vector.tensor_tensor(
                out_sb[:, :], raw_sb[:, :], expS_sb[:, :], mybir.AluOpType.mult
            )
            mainps_es.close()
            nc.sync.dma_start(out=out[:, :], in_=out_sb[:, :])

    nc.finalize()
    return nc


def main_loop(nc, mybir, xT_sb, xTj_sb, S16_sb, raw_sb, negI_sb, ones_sb,
              out_sb, dist_bufs, dump_bufs, ab_bufs, ablate=()):
    NAB = len(ab_bufs)
    if True:
            for j in range(JPC):
                dist = dist_bufs[j % 4]
                # dist = -S[k, i]; also the first touch of dist this
                # iteration, absorbing the WAR-vs-ACT(exp of j-2) wait.
                if "mms" not in ablate and "mm" not in ablate:
                    nc.tensor.matmul(
                        dist[:, :],
                        negI_sb[:, :],
                        S16_sb[:, :],
                        start=True,
                        stop=False,
                        skip_group_check=True,
                    )
                for c in range(NCHUNK):
                    ab = ab_bufs[(j * NCHUNK + c) % NAB]
                    # relu(x_i - x_j) = max(x_i, x_j) - x_j
                    if "ts" not in ablate:
                        # relu(x_i - x_j) = (x_i - x_j) max 0; const scalar2
                        # keeps the second DVE read port free for 2x_2p/4x.
                        s1 = (
                            0.5
                            if "tsconst" in ablate
                            else xTj_sb[0:CHUNK, c * JPC + j : c * JPC + j + 1]
                        )
                        nc.vector.tensor_scalar(
                            ab[:, :],
                            xT_sb[0:CHUNK, c * B : (c + 1) * B],
                            s1,
                            0.0,
                            mybir.AluOpType.subtract,
                            mybir.AluOpType.max,
                        )
                    # dist[32c+m, :] += 2 * sum_d ab[5m+d, :]
                    if "mm" not in ablate:
                        nc.tensor.matmul(
                            dist[32 * c : 32 * c + 32, :],
                            ones_sb[:, 0:32],
                            ab[:, :],
                            start=("mms" in ablate and c == 0),
                            stop=(c == NCHUNK - 1),
                            tile_position=(0, 32 * c),
                            skip_group_check=True,
                        )
                dump = dump_bufs[j % 2]
                # out_sb[:, j] = sum_i exp(-dist[:, i] - S16[:, j])
                if "exp" not in ablate:
                    nc.scalar.activation(
                        dump[:, :],
                        dist[:, :],
                        mybir.ActivationFunctionType.Exp,
                        bias=0.0,
                        scale=-1.0,
                        accum_out=(None if "noaccum" in ablate else raw_sb[:, j : j + 1]),
                    )


def _aux_consts():
    ob = np.zeros([CHUNK, 64], dtype=np.float16)
    for m in range(KPC):
        ob[5 * m : 5 * m + 5, m] = 2.0
        ob[5 * m : 5 * m + 5, 32 + m] = 1.0
    negI = (-np.eye(128)).astype(np.float16)
    return ob, negI


def make_in_maps(inputs, T):
    f16 = np.float16
    Tm = np.asarray(T, dtype=np.float32).astype(f16)
    ob, negI = _aux_consts()
    in_maps = []
    for c in range(NCORES):
        rolled = np.roll(np.asarray(inputs, dtype=np.float32), -JPC * c, axis=0)
        inTc = np.ascontiguousarray(rolled.T).astype(f16)
        in_maps.append(
            {
                "inT": inTc,
                "Tm": Tm,
                "onesd": ob,
                "negI": negI,
            }
        )
    return in_maps


def assemble_output(results):
    out = np.zeros([B, K], dtype=np.float32)
    for c in range(NCORES):
        arr = np.asarray(results[c]["out"], dtype=np.float32)  # [128, JPC]
        for cc in range(NCHUNK):
            out[JPC * c : JPC * (c + 1), KPC * cc : KPC * (cc + 1)] = arr[
                32 * cc : 32 * cc + KPC, :
            ].T
    return out


def kernel(inputs, T):
    from concourse.bass_utils import run_bass_kernel_spmd

    if "nc" not in _NC_CACHE:
        _NC_CACHE["nc"] = build_nc()
    nc = _NC_CACHE["nc"]
    in_maps = make_in_maps(inputs, T)
    res = run_bass_kernel_spmd(nc, in_maps, list(range(NCORES)))
    return assemble_output(res.results)


if __name__ == "__main__":
    sys.path.insert(0, "/root/problem")
    from reference import setup_inputs, reference

    inputs = setup_inputs()
    expected = np.asarray(reference(**inputs))
    actual = kernel(**{k: np.asarray(v) for k, v in inputs.items()})
    err = np.abs(actual - expected)
    rel = np.linalg.norm(actual - expected) / np.linalg.norm(expected)
    print(f"max abs err: {err.max():.3e}")
    print(f"Relative error: {rel:.3e}")



# revision 8
# speedup vs baseline: 1.5521x; 1.5521x over previous
"""
MinibatchDiscrimination kernel for 8x TRN2 NeuronCores (Bass/Tile).

Math:  x = inputs @ T  -> [B, K, D] with B=512, K=100, D=5
       out[a,k] = sum_b exp(-sum_d |x[a,k,d]-x[b,k,d]|)

v2 strategy — circulant-symmetric pair coverage (vs v1's full 512-wide rows):

  The distance matrix is symmetric, so each unordered pair {a, b} is computed
  ONCE globally. Core c owns global rows a = 64c+j (j=0..63) and, for each
  row, only the offsets delta = 0..256 (i.e. partners b = a+delta mod 512).
  Every pair {a, a+delta} (delta 1..255) appears exactly once this way;
  delta=256 pairs appear twice (once from each endpoint), handled by
  excluding delta=256 from the row's own accumulation (host subtracts the
  saved dumpc column) while the cross path keeps it.

  Each computed exp(-d(a, a+delta)) contributes to BOTH endpoint rows:
    - row a: ACT accum_out over the row's 257-wide window
    - row a+delta: accumulated into cross[k, j+delta] by the GPSIMD engine
  The host merges: out[64c+j] = raw[:,j] - dumpc[:,j], then scatters
  out[(64c+t) % 512] += cross[:, t] for t = 1..319.

  Per-core inputs are the batch-rolled inputs (roll by -64c), so the program
  is SPMD-identical; only columns 0..319 of the rolled x are needed, so the
  host sends inT = rolled.T[:, 0:320] and the projection matmuls run with
  free size 320 instead of 512.

  Identity used per (k,d):  |x_i - x_j| = 2*relu(x_i - x_j) - x_i + x_j,
  so  sum_d |..| = 2R - S_ki + S_kj  with S = sum_d x.  The -S_ki term is a
  single negI matmul into the dist psum (start of the accumulation group);
  the +S_kj term is constant per row and folded into the exp bias AP
  (bias = -S_kj, scale = -1), eliminating v1's final multiply.

  Per row j (engine assignment tuned against the CoreSim cost model, which
  charges matmuls as out_free_size x 0.42ns serialized on PE):
    PE : negI matmul (dist = -S window, FD=257) + 4 d-sum ones-matmuls
    DVE: 4x tensor_scalar relu chunks [125, 257] (fp16, 4x mode)
         + dumpc column save [128,1]
    ACT: exp(-dist - S_kj) -> dump (SBUF fp16) + accum_out -> raw psum
    POOL: cross[:, j+1..j+257] += dump[:, 1..257]   (the idle gpsimd engine
          takes the symmetric-partner accumulation)

  dist psum layout: partition 32c+m holds k=25c+m (m<25); host reassembles.
"""

import sys
import numpy as np

for _p in ("/opt/trn_rl_repo",):
    if _p not in sys.path:
        sys.path.insert(0, _p)

B = 512
F = 1024
K = 100
D = 5
KD = K * D  # 500
NCORES = 8
JPC = B // NCORES  # 64 output rows per core
NCHUNK = 4  # kd chunks of 125
CHUNK = KD // NCHUNK  # 125
KPC = K // NCHUNK  # 25 k's per chunk
FD = 257  # per-row window: delta = 0..256
W = JPC + FD - 1  # 320 columns of x needed per core

_NC_CACHE = {}


def build_nc():
    import contextlib

    import concourse.bass as bass
    import concourse.bacc as bacc
    import concourse.mybir as mybir
    from concourse.tile import TileContext

    nc = bacc.Bacc(None, target_bir_lowering=False, debug=True)

    inT = nc.declare_dram_parameter("inT", [F, W], mybir.dt.float16, isOutput=False)
    Tm = nc.declare_dram_parameter("Tm", [F, KD], mybir.dt.float16, isOutput=False)
    # [:, 0:32] 2.0-valued d-sum block, [:, 32:64] 1.0-valued d-sum block
    onesd = nc.declare_dram_parameter(
        "onesd", [CHUNK, 64], mybir.dt.float16, isOutput=False
    )
    negI = nc.declare_dram_parameter("negI", [128, 128], mybir.dt.float16, isOutput=False)
    raw_out = nc.declare_dram_parameter("raw", [128, JPC], mybir.dt.float32, isOutput=True)
    cross_out = nc.declare_dram_parameter(
        "cross", [128, W], mybir.dt.float32, isOutput=True
    )

    with TileContext(nc) as tc:
        with tc.tile_pool(name="persist", bufs=1) as pp:
            T_sb = pp.tile([128, 8 * KD], mybir.dt.float16, name="T_sb")
            inT_sb = pp.tile([128, 8 * W], mybir.dt.float16, name="inT_sb")
            ones_sb = pp.tile([CHUNK, 64], mybir.dt.float16, name="ones_sb")
            negI_sb = pp.tile([128, 128], mybir.dt.float16, name="negI_sb")
            xT_sb = pp.tile([128, NCHUNK * W], mybir.dt.float16, name="xT_sb")
            # f32 upcasts of the fp16 xT columns 0..JPC (tensor_scalar
            # per-partition scalars must be f32; upcasting from the fp16 xT
            # keeps the diagonal subtract exactly zero).
            xTj_sb = pp.tile([128, NCHUNK * JPC], mybir.dt.float32, name="xTj_sb")
            S16_sb = pp.tile([128, W], mybir.dt.float16, name="S16_sb")
            negS32_sb = pp.tile([128, JPC], mybir.dt.float32, name="negS32_sb")
            cross_sb = pp.tile([128, W], mybir.dt.float32, name="cross_sb")
            dumpc_sb = pp.tile([128, JPC], mybir.dt.float32, name="dumpc_sb")
            raw_sb = pp.tile([128, JPC], mybir.dt.float32, name="raw_sb")
            dump_bufs = [
                pp.tile([128, FD], mybir.dt.float16, name=f"dump{i}") for i in range(4)
            ]
            NAB = 8
            ab_bufs = [
                pp.tile([CHUNK, FD], mybir.dt.float16, name=f"ab{i}") for i in range(NAB)
            ]

            # warm the ACT exp table while DMAs run (table load ~1.3us)
            warm_sb = pp.tile([1, 1], mybir.dt.float32, name="warm_sb")
            nc.vector.memset(warm_sb[:, :], 0.0)
            nc.scalar.activation(
                warm_sb[:, :], warm_sb[:, :], mybir.ActivationFunctionType.Exp
            )
            nc.vector.memset(cross_sb[:, :], 0.0)

            # --- load inputs (one DMA per tensor; fewer DMAs = less SP/DGE
            # serialization in the cost model) ---
            nc.sync.dma_start(
                out=T_sb[:, :], in_=Tm[:, :].rearrange("(t p) c -> p t c", t=8)
            )
            nc.sync.dma_start(
                out=inT_sb[:, :], in_=inT[:, :].rearrange("(t p) c -> p t c", t=8)
            )
            nc.sync.dma_start(out=ones_sb[:, :], in_=onesd[:, :])
            nc.sync.dma_start(out=negI_sb[:, :], in_=negI[:, :])

            with tc.tile_pool(name="xtps", bufs=2, space="PSUM") as xtps:
                # --- xT chunks: xT[kd, i] via PE over f tiles, window W ---
                for c in range(NCHUNK):
                    xt_ps = xtps.tile([CHUNK, W], mybir.dt.float32, name="xt_ps")
                    for t in range(8):
                        nc.tensor.matmul(
                            xt_ps[:, :],
                            T_sb[:, t * KD + c * CHUNK : t * KD + (c + 1) * CHUNK],
                            inT_sb[:, t * W : (t + 1) * W],
                            start=(t == 0),
                            stop=(t == 7),
                        )
                    nc.vector.tensor_copy(
                        xT_sb[0:CHUNK, c * W : (c + 1) * W], xt_ps[:, :]
                    )
                    nc.vector.tensor_copy(
                        xTj_sb[0:CHUNK, c * JPC : (c + 1) * JPC],
                        xT_sb[0:CHUNK, c * W : c * W + JPC],
                    )

                # --- S[k, i] = sum_d x[i,k,d] from the fp16 xT (so the
                # diagonal cancels exactly), arranged at partitions 32c+m ---
                S_ps = xtps.tile([128, W], mybir.dt.float32, name="S_ps", bufs=1)
                for c in range(NCHUNK):
                    nc.tensor.matmul(
                        S_ps[32 * c : 32 * c + 32, :],
                        ones_sb[:, 32:64],
                        xT_sb[0:CHUNK, c * W : (c + 1) * W],
                        start=True,
                        stop=True,
                        tile_position=(0, 32 * c),
                    )
                nc.vector.tensor_copy(S16_sb[:, :], S_ps[:, :])
                nc.vector.tensor_scalar(
                    negS32_sb[:, :],
                    S_ps[:, 0:JPC],
                    -1.0,
                    None,
                    mybir.AluOpType.mult,
                )

            mainps_es = contextlib.ExitStack()
            mainps = mainps_es.enter_context(
                tc.tile_pool(name="mainps", bufs=1, space="PSUM")
            )
            # full-bank tiles (512 f32) so no two dist tiles share a psum
            # bank; only [:, 0:FD] is used
            dist_bufs = [
                mainps.tile([128, 512], mybir.dt.float32, name=f"dist{i}")
                for i in range(4)
            ]
            raw_ps = mainps.tile([128, JPC], mybir.dt.float32, name="raw_ps")

            # --- main loop over output rows ---
            for j in range(JPC):
                dist = dist_bufs[j % 4]
                # dist = -S[k, j..j+FD]; opens the psum accumulation group and
                # absorbs the WAR wait vs the exp of row j-4.
                nc.tensor.matmul(
                    dist[:, 0:FD],
                    negI_sb[:, :],
                    S16_sb[:, j : j + FD],
                    start=True,
                    stop=False,
                    skip_group_check=True,
                )
                for c in range(NCHUNK):
                    ab = ab_bufs[(j * NCHUNK + c) % NAB]
                    # relu(x_i - x_j) = (x_i - x_j) max 0
                    nc.vector.tensor_scalar(
                        ab[:, :],
                        xT_sb[0:CHUNK, c * W + j : c * W + j + FD],
                        xTj_sb[0:CHUNK, c * JPC + j : c * JPC + j + 1],
                        0.0,
                        mybir.AluOpType.subtract,
                        mybir.AluOpType.max,
                    )
                    # dist[32c+m, :] += 2 * sum_d ab[5m+d, :]
                    nc.tensor.matmul(
                        dist[32 * c : 32 * c + 32, 0:FD],
                        ones_sb[:, 0:32],
                        ab[:, :],
                        start=False,
                        stop=(c == NCHUNK - 1),
                        tile_position=(0, 32 * c),
                        skip_group_check=True,
                    )
                dump = dump_bufs[j % 4]
                # dump = exp(-dist - S_kj); raw[:, j] = sum_i dump
                nc.scalar.activation(
                    dump[:, :],
                    dist[:, 0:FD],
                    mybir.ActivationFunctionType.Exp,
                    bias=negS32_sb[:, j : j + 1],
                    scale=-1.0,
                    accum_out=raw_ps[:, j : j + 1],
                )
                # save the delta=256 column (host subtracts it from raw to
                # undo the double count of {a, a+256} pairs)
                nc.vector.tensor_copy(dumpc_sb[:, j : j + 1], dump[:, FD - 1 : FD])
                # symmetric partners: cross[k, j+delta] += dump[k, delta]
                nc.gpsimd.tensor_tensor(
                    cross_sb[:, j + 1 : j + FD],
                    cross_sb[:, j + 1 : j + FD],
                    dump[:, 1:FD],
                    mybir.AluOpType.add,
                )

            # own sums minus the delta=256 double-count, moved to SBUF for DMA
            nc.vector.tensor_tensor(
                raw_sb[:, :], raw_ps[:, :], dumpc_sb[:, :], mybir.AluOpType.subtract
            )
            mainps_es.close()
            nc.sync.dma_start(out=raw_out[:, :], in_=raw_sb[:, :])
            nc.sync.dma_start(out=cross_out[:, :], in_=cross_sb[:, :])

    nc.finalize()
    return nc


def _aux_consts():
    ob = np.zeros([CHUNK, 64], dtype=np.float16)
    for m in range(KPC):
        ob[5 * m : 5 * m + 5, m] = 2.0
        ob[5 * m : 5 * m + 5, 32 + m] = 1.0
    negI = (-np.eye(128)).astype(np.float16)
    return ob, negI


def make_in_maps(inputs, T):
    f16 = np.float16
    Tm = np.asarray(T, dtype=np.float32).astype(f16)
    ob, negI = _aux_consts()
    in_maps = []
    for c in range(NCORES):
        rolled = np.roll(np.asarray(inputs, dtype=np.float32), -JPC * c, axis=0)
        inTc = np.ascontiguousarray(rolled[0:W].T).astype(f16)
        in_maps.append(
            {
                "inT": inTc,
                "Tm": Tm,
                "onesd": ob,
                "negI": negI,
            }
        )
    return in_maps


def assemble_output(results):
    out = np.zeros([B, K], dtype=np.float32)
    for c in range(NCORES):
        own = np.asarray(results[c]["raw"], dtype=np.float32)  # [128, JPC]
        cross = np.asarray(results[c]["cross"], dtype=np.float32)  # [128, W]
        for cc in range(NCHUNK):
            ksl = slice(32 * cc, 32 * cc + KPC)
            kg = slice(KPC * cc, KPC * (cc + 1))
            # own rows: global rows 64c..64c+63
            out[JPC * c : JPC * (c + 1), kg] += own[ksl, :].T
            # cross rows: global rows (64c + t) % 512 for t = 1..W-1
            rows = (JPC * c + np.arange(1, W)) % B
            np.add.at(out, (rows[:, None], np.arange(KPC * cc, KPC * (cc + 1))[None, :]),
                      cross[ksl, 1:W].T)
    return out


def kernel(inputs, T):
    from concourse.bass_utils import run_bass_kernel_spmd

    if "nc" not in _NC_CACHE:
        _NC_CACHE["nc"] = build_nc()
    nc = _NC_CACHE["nc"]
    in_maps = make_in_maps(inputs, T)
    res = run_bass_kernel_spmd(nc, in_maps, list(range(NCORES)))
    return assemble_output(res.results)


if __name__ == "__main__":
    sys.path.insert(0, "/root/problem")
    from reference import setup_inputs, reference

    inputs = setup_inputs()
    expected = np.asarray(reference(**inputs))
    actual = kernel(**{k: np.asarray(v) for k, v in inputs.items()})
    err = np.abs(actual - expected)
    rel = np.linalg.norm(actual - expected) / np.linalg.norm(expected)
    print(f"max abs err: {err.max():.3e}")
    print(f"Relative error: {rel:.3e}")


# revision 11
# speedup vs baseline: 1.5870x; 1.0225x over previous
"""
MinibatchDiscrimination kernel for 8x TRN2 NeuronCores (Bass/Tile).

Math:  x = inputs @ T  -> [B, K, D] with B=512, K=100, D=5
       out[a,k] = sum_b exp(-sum_d |x[a,k,d]-x[b,k,d]|)

v2 strategy — circulant-symmetric pair coverage (vs v1's full 512-wide rows):

  The distance matrix is symmetric, so each unordered pair {a, b} is computed
  ONCE globally. Core c owns global rows a = 64c+j (j=0..63) and, for each
  row, only the offsets delta = 0..256 (i.e. partners b = a+delta mod 512).
  Every pair {a, a+delta} (delta 1..255) appears exactly once this way;
  delta=256 pairs appear twice (once from each endpoint), handled by
  excluding delta=256 from the row's own accumulation (host subtracts the
  saved dumpc column) while the cross path keeps it.

  Each computed exp(-d(a, a+delta)) contributes to BOTH endpoint rows:
    - row a: ACT accum_out over the row's 257-wide window
    - row a+delta: accumulated into cross[k, j+delta] by the GPSIMD engine
  The host merges: out[64c+j] = raw[:,j] - dumpc[:,j], then scatters
  out[(64c+t) % 512] += cross[:, t] for t = 1..319.

  Per-core inputs are the batch-rolled inputs (roll by -64c), so the program
  is SPMD-identical; only columns 0..319 of the rolled x are needed, so the
  host sends inT = rolled.T[:, 0:320] and the projection matmuls run with
  free size 320 instead of 512.

  Identity used per (k,d):  |x_i - x_j| = 2*relu(x_i - x_j) - x_i + x_j,
  so  sum_d |..| = 2R - S_ki + S_kj  with S = sum_d x.  The -S_ki term is a
  single negI matmul into the dist psum (start of the accumulation group);
  the +S_kj term is constant per row and folded into the exp bias AP
  (bias = -S_kj, scale = -1), eliminating v1's final multiply.

  Per row j (engine assignment tuned against the CoreSim cost model, which
  charges matmuls as out_free_size x 0.42ns serialized on PE):
    PE : negI matmul (dist = -S window, FD=257) + 4 d-sum ones-matmuls
    DVE: 4x tensor_scalar relu chunks [125, 257] (fp16, 4x mode)
         + dumpc column save [128,1]
    ACT: exp(-dist - S_kj) -> dump (SBUF fp16) + accum_out -> raw psum
    POOL: cross[:, j+1..j+257] += dump[:, 1..257]   (the idle gpsimd engine
          takes the symmetric-partner accumulation)

  dist psum layout: partition 32c+m holds k=25c+m (m<25); host reassembles.
"""

import sys
import numpy as np

for _p in ("/opt/trn_rl_repo",):
    if _p not in sys.path:
        sys.path.insert(0, _p)

B = 512
F = 1024
K = 100
D = 5
KD = K * D  # 500
NCORES = 8
JPC = B // NCORES  # 64 output rows per core
NCHUNK = 4  # kd chunks of 125
CHUNK = KD // NCHUNK  # 125
KPC = K // NCHUNK  # 25 k's per chunk
FD = 257  # per-row window: delta = 0..256
W = JPC + FD - 1  # 320 columns of x needed per core

_NC_CACHE = {}


def build_nc():
    import contextlib

    import concourse.bass as bass
    import concourse.bacc as bacc
    import concourse.mybir as mybir
    from concourse.tile import TileContext

    nc = bacc.Bacc(None, target_bir_lowering=False, debug=True)

    inT = nc.declare_dram_parameter("inT", [F, W], mybir.dt.float16, isOutput=False)
    Tm = nc.declare_dram_parameter("Tm", [F, KD], mybir.dt.float16, isOutput=False)
    # [:, 0:32] 2.0-valued d-sum block, [:, 32:64] 1.0-valued d-sum block
    onesd = nc.declare_dram_parameter(
        "onesd", [CHUNK, 64], mybir.dt.float16, isOutput=False
    )
    negI = nc.declare_dram_parameter("negI", [128, 128], mybir.dt.float16, isOutput=False)
    raw_out = nc.declare_dram_parameter("raw", [128, JPC], mybir.dt.float32, isOutput=True)
    cross_out = nc.declare_dram_parameter(
        "cross", [128, W], mybir.dt.float32, isOutput=True
    )

    with TileContext(nc) as tc:
        with tc.tile_pool(name="persist", bufs=1) as pp:
            T_sb = pp.tile([128, 8 * KD], mybir.dt.float16, name="T_sb")
            inT_sb = pp.tile([128, 8 * W], mybir.dt.float16, name="inT_sb")
            ones_sb = pp.tile([CHUNK, 64], mybir.dt.float16, name="ones_sb")
            negI_sb = pp.tile([128, 128], mybir.dt.float16, name="negI_sb")
            xT_sb = pp.tile([128, NCHUNK * W], mybir.dt.float16, name="xT_sb")
            # f32 upcasts of the fp16 xT columns 0..JPC (tensor_scalar
            # per-partition scalars must be f32; upcasting from the fp16 xT
            # keeps the diagonal subtract exactly zero).
            xTj_sb = pp.tile([128, NCHUNK * JPC], mybir.dt.float32, name="xTj_sb")
            S16_sb = pp.tile([128, W], mybir.dt.float16, name="S16_sb")
            negS32_sb = pp.tile([128, JPC], mybir.dt.float32, name="negS32_sb")
            cross_sb = pp.tile([128, W], mybir.dt.float32, name="cross_sb")
            raw_sb = pp.tile([128, JPC], mybir.dt.float32, name="raw_sb")
            dumpc_sb = pp.tile([128, JPC], mybir.dt.float32, name="dumpc_sb")
            dump_bufs = [
                pp.tile([128, FD], mybir.dt.float16, name=f"dump{i}") for i in range(4)
            ]
            NAB = 8
            ab_bufs = [
                pp.tile([CHUNK, FD], mybir.dt.float16, name=f"ab{i}") for i in range(NAB)
            ]

            # warm the ACT exp table while DMAs run (table load ~1.3us)
            warm_sb = pp.tile([1, 1], mybir.dt.float32, name="warm_sb")
            nc.vector.memset(warm_sb[:, :], 0.0)
            nc.scalar.activation(
                warm_sb[:, :], warm_sb[:, :], mybir.ActivationFunctionType.Exp
            )
            nc.vector.memset(cross_sb[:, :], 0.0)

            # --- load inputs (one DMA per tensor; fewer DMAs = less SP/DGE
            # serialization in the cost model) ---
            nc.sync.dma_start(out=ones_sb[:, :], in_=onesd[:, :])
            nc.sync.dma_start(out=negI_sb[:, :], in_=negI[:, :])
            for h in range(2):
                nc.sync.dma_start(
                    out=T_sb[:, h * 4 * KD : (h + 1) * 4 * KD],
                    in_=Tm[h * 512 : (h + 1) * 512, :].rearrange(
                        "(t p) c -> p t c", t=4
                    ),
                )
                nc.sync.dma_start(
                    out=inT_sb[:, h * 4 * W : (h + 1) * 4 * W],
                    in_=inT[h * 512 : (h + 1) * 512, :].rearrange(
                        "(t p) c -> p t c", t=4
                    ),
                )

            with tc.tile_pool(name="xtps", bufs=1, space="PSUM") as xtps:
                # --- xT chunks: xT[kd, i] via PE over f tiles, window W.
                # t-outer order so the first f-tiles' matmuls overlap the
                # second half of the input DMAs. ---
                xt_ps = [
                    xtps.tile([CHUNK, W], mybir.dt.float32, name=f"xt_ps{c}")
                    for c in range(NCHUNK)
                ]
                for t in range(8):
                    for c in range(NCHUNK):
                        nc.tensor.matmul(
                            xt_ps[c][:, :],
                            T_sb[:, t * KD + c * CHUNK : t * KD + (c + 1) * CHUNK],
                            inT_sb[:, t * W : (t + 1) * W],
                            start=(t == 0),
                            stop=(t == 7),
                            skip_group_check=True,
                        )
                for c in range(NCHUNK):
                    nc.vector.tensor_copy(
                        xT_sb[0:CHUNK, c * W : (c + 1) * W], xt_ps[c][:, :]
                    )
                    nc.vector.tensor_copy(
                        xTj_sb[0:CHUNK, c * JPC : (c + 1) * JPC],
                        xT_sb[0:CHUNK, c * W : c * W + JPC],
                    )

                # --- S[k, i] = sum_d x[i,k,d] from the fp16 xT (so the
                # diagonal cancels exactly), arranged at partitions 32c+m ---
                S_ps = xtps.tile([128, W], mybir.dt.float32, name="S_ps", bufs=1)
                for c in range(NCHUNK):
                    nc.tensor.matmul(
                        S_ps[32 * c : 32 * c + 32, :],
                        ones_sb[:, 32:64],
                        xT_sb[0:CHUNK, c * W : (c + 1) * W],
                        start=True,
                        stop=True,
                        tile_position=(0, 32 * c),
                    )
                nc.vector.tensor_copy(S16_sb[:, :], S_ps[:, :])
                nc.vector.tensor_scalar(
                    negS32_sb[:, :],
                    S_ps[:, 0:JPC],
                    -1.0,
                    None,
                    mybir.AluOpType.mult,
                )

            mainps_es = contextlib.ExitStack()
            mainps = mainps_es.enter_context(
                tc.tile_pool(name="mainps", bufs=1, space="PSUM")
            )
            # full-bank tiles (512 f32) so no two dist tiles share a psum
            # bank; only [:, 0:FD] is used
            dist_bufs = [
                mainps.tile([128, 512], mybir.dt.float32, name=f"dist{i}")
                for i in range(4)
            ]
            raw_ps = mainps.tile([128, JPC], mybir.dt.float32, name="raw_ps")

            # --- main loop over output rows ---
            for j in range(JPC):
                dist = dist_bufs[j % 4]
                # dist = -S[k, j..j+FD]; opens the psum accumulation group and
                # absorbs the WAR wait vs the exp of row j-4.
                nc.tensor.matmul(
                    dist[:, 0:FD],
                    negI_sb[:, :],
                    S16_sb[:, j : j + FD],
                    start=True,
                    stop=False,
                    skip_group_check=True,
                )
                for c in range(NCHUNK):
                    ab = ab_bufs[(j * NCHUNK + c) % NAB]
                    # relu(x_i - x_j) = (x_i - x_j) max 0
                    nc.vector.tensor_scalar(
                        ab[:, :],
                        xT_sb[0:CHUNK, c * W + j : c * W + j + FD],
                        xTj_sb[0:CHUNK, c * JPC + j : c * JPC + j + 1],
                        0.0,
                        mybir.AluOpType.subtract,
                        mybir.AluOpType.max,
                    )
                    # dist[32c+m, :] += 2 * sum_d ab[5m+d, :]
                    nc.tensor.matmul(
                        dist[32 * c : 32 * c + 32, 0:FD],
                        ones_sb[:, 0:32],
                        ab[:, :],
                        start=False,
                        stop=(c == NCHUNK - 1),
                        tile_position=(0, 32 * c),
                        skip_group_check=True,
                    )
                dump = dump_bufs[j % 4]
                # dump = exp(-dist - S_kj); raw[:, j] = sum_i dump
                nc.scalar.activation(
                    dump[:, :],
                    dist[:, 0:FD],
                    mybir.ActivationFunctionType.Exp,
                    bias=negS32_sb[:, j : j + 1],
                    scale=-1.0,
                    accum_out=raw_ps[:, j : j + 1],
                )
                # save the delta=256 column (subtracted from raw at the end
                # to undo the double count of {a, a+256} pairs)
                nc.vector.tensor_copy(dumpc_sb[:, j : j + 1], dump[:, FD - 1 : FD])
                # symmetric partners: cross[k, j+delta] += dump[k, delta]
                nc.gpsimd.tensor_tensor(
                    cross_sb[:, j + 1 : j + FD],
                    cross_sb[:, j + 1 : j + FD],
                    dump[:, 1:FD],
                    mybir.AluOpType.add,
                )

            # own sums minus the delta=256 double-count, moved to SBUF for DMA
            nc.vector.tensor_tensor(
                raw_sb[:, :], raw_ps[:, :], dumpc_sb[:, :], mybir.AluOpType.subtract
            )
            mainps_es.close()
            nc.sync.dma_start(out=raw_out[:, :], in_=raw_sb[:, :])
            nc.sync.dma_start(out=cross_out[:, :], in_=cross_sb[:, :])

    nc.finalize()
    return nc


def _aux_consts():
    ob = np.zeros([CHUNK, 64], dtype=np.float16)
    for m in range(KPC):
        ob[5 * m : 5 * m + 5, m] = 2.0
        ob[5 * m : 5 * m + 5, 32 + m] = 1.0
    negI = (-np.eye(128)).astype(np.float16)
    return ob, negI


def make_in_maps(inputs, T):
    f16 = np.float16
    Tm = np.asarray(T, dtype=np.float32).astype(f16)
    ob, negI = _aux_consts()
    in_maps = []
    for c in range(NCORES):
        rolled = np.roll(np.asarray(inputs, dtype=np.float32), -JPC * c, axis=0)
        inTc = np.ascontiguousarray(rolled[0:W].T).astype(f16)
        in_maps.append(
            {
                "inT": inTc,
                "Tm": Tm,
                "onesd": ob,
                "negI": negI,
            }
        )
    return in_maps


def assemble_output(results):
    out = np.zeros([B, K], dtype=np.float32)
    for c in range(NCORES):
        own = np.asarray(results[c]["raw"], dtype=np.float32)  # [128, JPC]
        cross = np.asarray(results[c]["cross"], dtype=np.float32)  # [128, W]
        for cc in range(NCHUNK):
            ksl = slice(32 * cc, 32 * cc + KPC)
            kg = slice(KPC * cc, KPC * (cc + 1))
            # own rows: global rows 64c..64c+63
            out[JPC * c : JPC * (c + 1), kg] += own[ksl, :].T
            # cross rows: global rows (64c + t) % 512 for t = 1..W-1
            rows = (JPC * c + np.arange(1, W)) % B
            np.add.at(out, (rows[:, None], np.arange(KPC * cc, KPC * (cc + 1))[None, :]),
                      cross[ksl, 1:W].T)
    return out


def kernel(inputs, T):
    from concourse.bass_utils import run_bass_kernel_spmd

    if "nc" not in _NC_CACHE:
        _NC_CACHE["nc"] = build_nc()
    nc = _NC_CACHE["nc"]
    in_maps = make_in_maps(inputs, T)
    res = run_bass_kernel_spmd(nc, in_maps, list(range(NCORES)))
    return assemble_output(res.results)


if __name__ == "__main__":
    sys.path.insert(0, "/root/problem")
    from reference import setup_inputs, reference

    inputs = setup_inputs()
    expected = np.asarray(reference(**inputs))
    actual = kernel(**{k: np.asarray(v) for k, v in inputs.items()})
    err = np.abs(actual - expected)
    rel = np.linalg.norm(actual - expected) / np.linalg.norm(expected)
    print(f"max abs err: {err.max():.3e}")
    print(f"Relative error: {rel:.3e}")
